# revision 1
# baseline (speedup 1.0000x reference)
"""Trainium2 Bass kernel for nn_MAB_17471926960685 (dense_transformer).

Sharding: token-parallel over N. Each of 8 cores takes a 256-token slice of N
(both batches); attention keys are full (K/V computed replicated from Y).
No collectives.

Scores are computed transposed (keys on partitions, tokens on free axis)
against host-pretransposed bf16 encoding tables:
  - add_enc/16 is accumulated into the QK PSUM via an identity*(1/16) matmul
  - exp on ScalarE doubles as the PSUM->SBUF evacuation (bf16 out)
  - softmax denominators via ones-column matmuls on PE (sum over partitions)
  - mult_enc applied on VectorE in bf16 (2x mode)
  - 1/den broadcast onto MH_raw^T via selection-matrix matmuls
"""

import math
import sys

import numpy as np
import ml_dtypes

sys.path.insert(0, "/opt/trn_rl_repo")

import concourse.bass as bass
import concourse.mybir as mybir
import concourse.tile as tile
from concourse import bacc
from concourse.masks import make_identity
from concourse.bass_utils import run_bass_kernel_spmd

B, N, D, H = 2, 2048, 256, 8
DS = D // H          # 32
NCORES = 8
NL = N // NCORES     # 256 tokens per core per batch
TOK = B * NL         # 512 tokens per core
NKT = N // 128       # 16 key tiles
EPS = 1e-5
F32 = mybir.dt.float32
BF16 = mybir.dt.bfloat16
AX = mybir.AluOpType
AF = mybir.ActivationFunctionType


def _ln_apply(nc, pool, x_ap, g_bc, b_bc, out_ap):
    """LayerNorm rows of x_ap [128, D] -> out_ap (f32)."""
    stats = pool.tile([128, 6], F32, tag="ln_stats")
    mv = pool.tile([128, 2], F32, tag="ln_mv")
    nc.vector.bn_stats(out=stats, in_=x_ap)
    nc.vector.bn_aggr(out=mv, in_=stats)
    eps_t = pool.tile([128, 1], F32, tag="ln_eps")
    nc.vector.memset(eps_t, EPS)
    std = pool.tile([128, 1], F32, tag="ln_std")
    nc.scalar.activation(std, mv[:, 1:2], AF.Sqrt, bias=eps_t)
    rstd = pool.tile([128, 1], F32, tag="ln_rstd")
    nc.vector.reciprocal(rstd, std)
    xn = pool.tile([128, D], F32, tag="ln_xn")
    nc.vector.tensor_scalar(xn, x_ap, mv[:, 0:1], rstd, AX.subtract, AX.mult)
    nc.vector.tensor_tensor(xn, xn, g_bc, AX.mult)
    nc.vector.tensor_tensor(out_ap, xn, b_bc, AX.add)


def build_kernel(gelu_af=AF.Gelu_apprx_tanh):
    nc = bacc.Bacc()
    P = {}
    for name, shape in [
        ("Xs", [B, NL, D]),
        ("bq", [D]), ("bk", [D]), ("bv", [D]), ("bmix", [D]),
        ("g0", [D]), ("b0", [D]), ("g1", [D]), ("b1", [D]),
    ]:
        P[name] = nc.declare_dram_parameter(name, shape, F32, isOutput=False)
    for name, shape in [
        ("Y", [B, N, D]),
        ("Wq", [D, D]), ("Wk", [D, D]), ("Wv", [D, D]), ("Wmix", [D, D]),
        ("wi0", [4 * D, D]), ("wi1", [4 * D, D]), ("wo", [D, 4 * D]),
        ("addT", [H, N, NL]), ("multT", [H, N, NL]),
    ]:
        P[name] = nc.declare_dram_parameter(name, shape, BF16, isOutput=False)
    out_ext = nc.declare_dram_parameter("out", [B, NL, D], F32, isOutput=True)

    with tile.TileContext(nc) as tc:
        with tc.tile_pool(name="persist", bufs=1) as pp, \
             tc.tile_pool(name="wload", bufs=2) as wlp, \
             tc.tile_pool(name="ln", bufs=2) as lnp, \
             tc.tile_pool(name="enc", bufs=2) as encp, \
             tc.tile_pool(name="pa", bufs=2) as pap, \
             tc.tile_pool(name="ytp", bufs=1) as ytp, \
             tc.tile_pool(name="psA", bufs=2, space="PSUM") as psA, \
             tc.tile_pool(name="psB", bufs=2, space="PSUM") as psB, \
             tc.tile_pool(name="psS", bufs=2, space="PSUM") as psS, \
             tc.tile_pool(name="psM", bufs=1, space="PSUM") as psM, \
             tc.tile_pool(name="psD", bufs=1, space="PSUM") as psD:

            # ---------- constants ----------
            id16 = pp.tile([128, 128], BF16)
            make_identity(nc, id16)
            nc.vector.tensor_scalar_mul(id16, id16, 1.0 / 16.0)
            ones_col = pp.tile([128, 1], BF16)
            nc.vector.memset(ones_col, 1.0)
            ones_row = pp.tile([1, TOK], F32)
            nc.vector.memset(ones_row, 1.0)

            brow = {}
            for name in ("bq", "bk", "bv"):
                t = pp.tile([1, D], F32, tag=f"brow_{name}")
                nc.sync.dma_start(out=t,
                                  in_=P[name][:].rearrange("(o d) -> o d", o=1))
                brow[name] = t
            bcast = {}
            for name in ("g0", "b0", "g1", "b1", "bmix"):
                t = pp.tile([128, D], F32, tag=f"bc_{name}")
                ap = P[name][:].rearrange("(o d) -> o d", o=1)
                bap = bass.AP(tensor=ap.tensor, offset=ap.offset,
                              ap=[[0, 128], ap.ap[1]])
                nc.sync.dma_start(out=t, in_=bap)
                bcast[name] = t
            mask_all = pp.tile([1, 4 * 128], BF16)
            nc.vector.memset(mask_all, 0.0)
            for j in range(4):
                nc.vector.memset(mask_all[0:1, j * 128 + 32 * j:
                                          j * 128 + 32 * j + 32], 1.0)

            # ---------- weights: load + PE-transpose -> W^T bf16 ----------
            def load_wT(hnd, rows, cols, tagp):
                """DRAM [rows, cols] -> W^T bf16 tiles: cols//128 tiles of
                [128 (col block), rows]."""
                tiles = [pp.tile([128, rows], BF16, tag=f"{tagp}{i}", name=f"{tagp}{i}") for i in range(cols // 128)]
                for ri in range(rows // 128):
                    w_n = wlp.tile([128, cols], BF16, tag="wstage")
                    nc.sync.dma_start(
                        out=w_n,
                        in_=hnd[:].rearrange("(t p) c -> t p c", p=128)[ri])
                    for co in range(cols // 128):
                        nc.sync.dma_start(
                            out=tiles[co][:, ri * 128:(ri + 1) * 128],
                            in_=w_n[:, co * 128:(co + 1) * 128],
                            transpose=True)
                return tiles

            WqT = load_wT(P["Wq"], D, D, "WqT")        # 2 x [128(dq), 256(de)]
            WkT = load_wT(P["Wk"], D, D, "WkT")
            WvT = load_wT(P["Wv"], D, D, "WvT")
            WmixT = load_wT(P["Wmix"], D, D, "WmixT")
            wi0T = load_wT(P["wi0"], 4 * D, D, "wi0T")  # 2 x [128(do), 1024(u)]
            wi1T = load_wT(P["wi1"], 4 * D, D, "wi1T")
            woT = load_wT(P["wo"], D, 4 * D, "woT")    # 8 x [128(u), 256(do)]

            # ---------- phase 1: LN0(X rows); Q^T (scores) and Q_N (residual) --
            lnx_n = []
            for b in range(B):
                x_n = wlp.tile([128, 2 * D], F32, tag="xload")
                nc.sync.dma_start(
                    out=x_n.rearrange("p (s d) -> p s d", s=2),
                    in_=P["Xs"][b].rearrange("(s p) d -> p s d", p=128))
                for s in range(2):
                    o = pp.tile([128, D], F32, tag=f"lnx{b}{s}")
                    _ln_apply(nc, lnp, x_n[:, s * D:(s + 1) * D],
                              bcast["g0"], bcast["b0"], o)
                    lnx_n.append(o)                      # tt = b*2 + s
            lnxT = [pp.tile([128, TOK], BF16, tag=f"lnxT{i}", name=f"lnxT{i}") for i in range(2)]
            for tt in range(4):
                lnxb = pap.tile([128, D], BF16, tag="lnxb")
                nc.scalar.copy(lnxb, lnx_n[tt])
                for dq in range(2):
                    nc.sync.dma_start(
                        out=lnxT[dq][:, tt * 128:(tt + 1) * 128],
                        in_=lnxb[:, dq * 128:(dq + 1) * 128], transpose=True)

            # Q^T/16 bf16: 4 tiles [64, TOK] (2 heads each at bases 0/32)
            qsT = [pp.tile([64, TOK], BF16, tag=f"qsT{i}", name=f"qsT{i}") for i in range(4)]
            for j in range(4):
                ps = psB.tile([64, TOK], F32, tag="big")
                for kq in range(2):
                    nc.tensor.matmul(ps, WqT[kq][:, j * 64:(j + 1) * 64],
                                     lnxT[kq], start=(kq == 0), stop=False)
                nc.tensor.matmul(ps, brow["bq"][0:1, j * 64:(j + 1) * 64],
                                 ones_row, start=False, stop=True)
                nc.scalar.activation(qsT[j], ps, AF.Copy, scale=1.0 / 16.0)
            # Q_N f32 (residual, includes bq): out[tok block, de]
            qN = []
            for tt in range(4):
                ps = psB.tile([128, D], F32, tag="big")
                for kq in range(2):
                    nc.tensor.matmul(ps, lnxT[kq][:, tt * 128:(tt + 1) * 128],
                                     WqT[kq], start=(kq == 0), stop=False)
                nc.tensor.matmul(ps, ones_row[0:1, 0:128], brow["bq"],
                                 start=False, stop=True)
                t = pp.tile([128, D], F32, tag=f"qN{tt}")
                nc.scalar.copy(t, ps)
                qN.append(t)

            # ---------- phase 2: Y^T; K^T bf16; V_N bf16 ----------
            kT = []   # [b][de block] -> [128, N] bf16
            vN = []   # [b] -> [128, NKT*256] bf16 (key block kt at cols kt*256)
            for b in range(B):
                yT = [ytp.tile([128, N], BF16, tag=f"yT{i}", name=f"yT{i}") for i in range(2)]
                yn = ytp.tile([128, NKT * D], BF16, tag="yn")
                nc.sync.dma_start(
                    out=yn.rearrange("p (nt d) -> p nt d", nt=NKT),
                    in_=P["Y"][b].rearrange("(nt p) d -> p nt d", p=128))
                for nt in range(NKT):
                    for dd in range(2):
                        nc.sync.dma_start(
                            out=yT[dd][:, nt * 128:(nt + 1) * 128],
                            in_=yn[:, nt * D + dd * 128:nt * D + (dd + 1) * 128],
                            transpose=True)
                ktb = []
                for j in range(4):
                    t = pp.tile([64, N], BF16, tag=f"kT{b}{j}", name=f"kT{b}{j}")
                    for ch in range(N // 512):
                        ps = psB.tile([64, 512], F32, tag="big")
                        sl = slice(ch * 512, (ch + 1) * 512)
                        for kd in range(2):
                            nc.tensor.matmul(
                                ps, WkT[kd][:, j * 64:(j + 1) * 64],
                                yT[kd][:, sl], start=(kd == 0), stop=False)
                        nc.tensor.matmul(
                            ps, brow["bk"][0:1, j * 64:(j + 1) * 64],
                            ones_row[0:1, 0:512], start=False, stop=True)
                        nc.scalar.copy(t[:, sl], ps)
                    ktb.append(t)
                kT.append(ktb)
                vb = pp.tile([128, NKT * D], BF16, tag=f"vN{b}")
                for kt in range(NKT):
                    ps = psB.tile([128, D], F32, tag="big")
                    for kd in range(2):
                        nc.tensor.matmul(
                            ps, yT[kd][:, kt * 128:(kt + 1) * 128], WvT[kd],
                            start=(kd == 0), stop=False)
                    nc.tensor.matmul(ps, ones_row[0:1, 0:128], brow["bv"],
                                     start=False, stop=True)
                    nc.scalar.copy(vb[:, kt * D:(kt + 1) * D], ps)
                vN.append(vb)

            # ---------- phase 3: attention ----------
            recip_wide = pp.tile([1, 16 * NL], BF16)
            mhT = [pp.tile([128, TOK], BF16, tag=f"mhT{i}", name=f"mhT{i}") for i in range(2)]
            for h in range(H):
                at_h = encp.tile([128, NKT * NL], BF16, tag="addT")
                nc.sync.dma_start(
                    out=at_h.rearrange("p (kt t) -> p kt t", kt=NKT),
                    in_=P["addT"][h].rearrange("(kt p) t -> p kt t", p=128))
                mt_h = encp.tile([128, NKT * NL], BF16, tag="multT")
                nc.sync.dma_start(
                    out=mt_h.rearrange("p (kt t) -> p kt t", kt=NKT),
                    in_=P["multT"][h].rearrange("(kt p) t -> p kt t", p=128))
                g, r = h // 4, 32 * (h % 4)
                j, r2 = h // 2, 32 * (h % 2)
                for b in range(B):
                    ps_mh = psM.tile([32, NL], F32, tag="mh")
                    ps_den = psD.tile([1, NL], F32, tag="den")
                    for kt in range(NKT):
                        ps_s = psS.tile([128, NL], F32, tag="s")
                        nc.tensor.matmul(
                            ps_s,
                            kT[b][j][r2:r2 + DS, kt * 128:(kt + 1) * 128],
                            qsT[j][r2:r2 + DS, b * NL:(b + 1) * NL],
                            start=True, stop=False)
                        nc.tensor.matmul(
                            ps_s, id16, at_h[:, kt * NL:(kt + 1) * NL],
                            start=False, stop=True)
                        pt = pap.tile([128, NL], BF16, tag="pt")
                        nc.scalar.activation(pt, ps_s, AF.Exp)
                        nc.tensor.matmul(ps_den, ones_col, pt,
                                         start=(kt == 0), stop=(kt == NKT - 1))
                        at = pap.tile([128, NL], BF16, tag="at")
                        nc.vector.tensor_tensor(
                            at, pt, mt_h[:, kt * NL:(kt + 1) * NL], AX.mult)
                        nc.tensor.matmul(
                            ps_mh,
                            vN[b][:, kt * D + r + 128 * g:
                                  kt * D + r + 128 * g + DS],
                            at, start=(kt == 0), stop=(kt == NKT - 1))
                    q = b * 8 + h
                    rcp = lnp.tile([1, NL], F32, tag="rcp")
                    nc.vector.reciprocal(rcp, ps_den)
                    nc.vector.tensor_copy(
                        recip_wide[0:1, q * NL:(q + 1) * NL], rcp)
                    nc.scalar.copy(mhT[g][r:r + DS, b * NL:(b + 1) * NL], ps_mh)

            # ---------- phase 4: 1/den, mix, residual ----------
            rb = [pp.tile([128, TOK], BF16, tag=f"rb{i}", name=f"rb{i}") for i in range(2)]
            for t in range(2):
                for b in range(B):
                    ps = psA.tile([128, NL], F32, tag="sm")
                    for hh in range(4):
                        q = b * 8 + 4 * t + hh
                        nc.tensor.matmul(
                            ps, mask_all[0:1, hh * 128:(hh + 1) * 128],
                            recip_wide[0:1, q * NL:(q + 1) * NL],
                            start=(hh == 0), stop=(hh == 3))
                    nc.scalar.copy(rb[t][:, b * NL:(b + 1) * NL], ps)
            mhsT = [pp.tile([128, TOK], BF16, tag=f"mhsT{i}", name=f"mhsT{i}") for i in range(2)]
            for t in range(2):
                nc.vector.tensor_tensor(mhsT[t], mhT[t], rb[t], AX.mult)
            mxT = [pp.tile([128, TOK], BF16, tag=f"mxT{i}", name=f"mxT{i}") for i in range(2)]
            for t in range(2):
                ps = psB.tile([128, TOK], F32, tag="big")
                for kd in range(2):
                    nc.tensor.matmul(ps, WmixT[kd][:, t * 128:(t + 1) * 128],
                                     mhsT[kd], start=(kd == 0),
                                     stop=(kd == 1))
                nc.scalar.copy(mxT[t], ps)
            hid = []
            for tt in range(4):
                t = pp.tile([128, D], F32, tag=f"hid{tt}")
                hid.append(t)
            for tt in range(4):
                for t in range(2):
                    mixn = pap.tile([128, 128], BF16, tag="mixn")
                    nc.sync.dma_start(out=mixn,
                                      in_=mxT[t][:, tt * 128:(tt + 1) * 128],
                                      transpose=True)
                    sl = slice(t * 128, (t + 1) * 128)
                    nc.vector.tensor_tensor(hid[tt][:, sl], mixn, qN[tt][:, sl],
                                            AX.add)
                    nc.vector.tensor_tensor(hid[tt][:, sl], hid[tt][:, sl],
                                            bcast["bmix"][:, sl], AX.add)

            # ---------- phase 5: LN1 + FFN + residual out ----------
            hrT = [pp.tile([128, TOK], BF16, tag=f"hrT{i}", name=f"hrT{i}") for i in range(2)]
            for tt in range(4):
                hr = lnp.tile([128, D], F32, tag="hr")
                _ln_apply(nc, lnp, hid[tt], bcast["g1"], bcast["b1"], hr)
                hrb = pap.tile([128, D], BF16, tag="hrb")
                nc.scalar.copy(hrb, hr)
                for dd in range(2):
                    nc.sync.dma_start(
                        out=hrT[dd][:, tt * 128:(tt + 1) * 128],
                        in_=hrb[:, dd * 128:(dd + 1) * 128], transpose=True)
            ffin = []
            for m in range(8):
                ps0 = psB.tile([128, TOK], F32, tag="big")
                ps1 = psB.tile([128, TOK], F32, tag="big")
                for kd in range(2):
                    nc.tensor.matmul(ps0, wi0T[kd][:, m * 128:(m + 1) * 128],
                                     hrT[kd], start=(kd == 0), stop=(kd == 1))
                    nc.tensor.matmul(ps1, wi1T[kd][:, m * 128:(m + 1) * 128],
                                     hrT[kd], start=(kd == 0), stop=(kd == 1))
                gt = pap.tile([128, TOK], BF16, tag="gelu")
                nc.scalar.activation(gt, ps0, gelu_af)
                ut = pap.tile([128, TOK], BF16, tag="u1c")
                nc.scalar.copy(ut, ps1)
                ft = pp.tile([128, TOK], BF16, tag=f"ffin{m}")
                nc.vector.tensor_tensor(ft, gt, ut, AX.mult)
                ffin.append(ft)
            for t in range(2):
                ps = psB.tile([128, TOK], F32, tag="big")
                for ku in range(8):
                    nc.tensor.matmul(ps, woT[ku][:, t * 128:(t + 1) * 128],
                                     ffin[ku], start=(ku == 0), stop=(ku == 7))
                fft = pap.tile([128, TOK], BF16, tag="ffT")
                nc.scalar.copy(fft, ps)
                for tt in range(4):
                    ffn = pap.tile([128, 128], BF16, tag="ffn")
                    nc.sync.dma_start(out=ffn,
                                      in_=fft[:, tt * 128:(tt + 1) * 128],
                                      transpose=True)
                    o = pap.tile([128, 128], F32, tag="outN")
                    nc.vector.tensor_tensor(
                        o, ffn, hid[tt][:, t * 128:(t + 1) * 128], AX.add)
                    nc.sync.dma_start(
                        out=out_ext[tt // 2].rearrange(
                            "(s p) d -> s p d", p=128)[tt % 2][:, t * 128:(t + 1) * 128],
                        in_=o)
    nc.finalize()
    return nc


_SEL = None


def _selmask_np():
    global _SEL
    if _SEL is None:
        s = np.zeros((16, 2 * B * 128), np.float32)
        for t in range(2):
            for b in range(B):
                for p in range(128):
                    s[b * 8 + t * 4 + p // 32, (t * B + b) * 128 + p] = 1.0
        _SEL = s
    return _SEL


def prepare_in_maps(inputs):
    bf = ml_dtypes.bfloat16
    X = np.asarray(inputs["X"], np.float32)
    Yf = np.asarray(inputs["Y"], np.float32)
    add_enc = np.asarray(inputs["add_enc"], np.float32)
    mult_enc = np.asarray(inputs["mult_enc"], np.float32)
    common = {k: np.asarray(inputs[k], np.float32)
              for k in ("bq", "bk", "bv", "bmix", "g0", "b0", "g1", "b1")}
    for k in ("Wq", "Wk", "Wv", "Wmix", "wi0", "wi1", "wo"):
        common[k] = np.asarray(inputs[k], np.float32).astype(bf)
    common["Y"] = Yf.astype(bf)
    in_maps = []
    for c in range(NCORES):
        sl = slice(c * NL, (c + 1) * NL)
        m = dict(common)
        m["Xs"] = np.ascontiguousarray(X[:, sl, :])
        m["addT"] = np.ascontiguousarray(
            add_enc[:, sl, :].transpose(0, 2, 1)).astype(bf)
        m["multT"] = np.ascontiguousarray(
            mult_enc[:, sl, :].transpose(0, 2, 1)).astype(bf)
        in_maps.append(m)
    return in_maps


def kernel(**inputs):
    in_maps = prepare_in_maps(inputs)
    nc = build_kernel()
    res = run_bass_kernel_spmd(nc, in_maps, list(range(NCORES)))
    out = np.empty((B, N, D), np.float32)
    for c in range(NCORES):
        out[:, c * NL:(c + 1) * NL, :] = res.results[c]["out"]
    return out


if __name__ == "__main__":
    nc = build_kernel()
    print("build OK")



# revision 27
# speedup vs baseline: 2.1785x; 2.1785x over previous
"""Trainium2 Bass kernel for nn_MAB_17471926960685 (dense_transformer).

Sharding: token-parallel over N. Each of 8 cores takes a 256-token slice of N
(both batches); K/V are computed replicated from the full Y. No collectives.

v2 design notes:
  - All weights and Y are pre-transposed on host (no DMA transposes).
  - add_enc is folded on host: ET = exp(add_enc/16), MT = mult_enc * ET.
    Scores then need only exp(QK/16) on ACT; the enc-add matmul pass is gone.
  - Attention per (head, batch) group g: PE computes 16 score matmuls
    (keys on partitions, tokens free); ACT evacuates PSUM pairs with
    exp(scale=1/16) to fp16; DVE forms at = MT*pt (PV weights) and
    ept = ET*pt (denominator integrand); PE then accumulates den (ones
    matmul) and MH (V matmul) for the PREVIOUS group, interleaved between
    score pairs so the PE never waits on the ACT/DVE chain.
  - fp16 everywhere for 2-byte tensors (better mantissa than bf16, same
    PE/DVE speed); f32 for residual/LN paths.
"""

import math
import sys

import numpy as np
import ml_dtypes

sys.path.insert(0, "/opt/trn_rl_repo")

import concourse.bass as bass
import concourse.mybir as mybir
import concourse.tile as tile
from concourse import bacc
from concourse.masks import make_identity
from concourse.bass_utils import run_bass_kernel_spmd

B, N, D, H = 2, 2048, 256, 8
DS = D // H          # 32
NCORES = 8
NL = N // NCORES     # 256 tokens per core per batch
TOK = B * NL         # 512 tokens per core
NKT = N // 128       # 16 key tiles
EPS = 1e-5
F32 = mybir.dt.float32
F16 = mybir.dt.float16
AX = mybir.AluOpType
AF = mybir.ActivationFunctionType


def _ln_apply(nc, pool, x_ap, g_bc, b_bc, out_ap):
    """LayerNorm rows of x_ap [128, D] -> out_ap (any dtype)."""
    stats = pool.tile([128, 6], F32, tag="ln_stats")
    mv = pool.tile([128, 2], F32, tag="ln_mv")
    nc.vector.bn_stats(out=stats, in_=x_ap)
    nc.vector.bn_aggr(out=mv, in_=stats)
    eps_t = pool.tile([128, 1], F32, tag="ln_eps")
    nc.vector.memset(eps_t, EPS)
    std = pool.tile([128, 1], F32, tag="ln_std")
    nc.scalar.activation(std, mv[:, 1:2], AF.Sqrt, bias=eps_t)
    rstd = pool.tile([128, 1], F32, tag="ln_rstd")
    nc.vector.reciprocal(rstd, std)
    xn = pool.tile([128, D], F32, tag="ln_xn")
    nc.vector.tensor_scalar(xn, x_ap, mv[:, 0:1], rstd, AX.subtract, AX.mult)
    nc.vector.tensor_tensor(xn, xn, g_bc, AX.mult)
    nc.vector.tensor_tensor(out_ap, xn, b_bc, AX.add)


def build_kernel(gelu_af=AF.Gelu_apprx_tanh):
    nc = bacc.Bacc()
    P = {}
    for name, shape in [
        ("Xs", [B, NL, D]),
        ("bq", [D]), ("bk", [D]), ("bv", [D]), ("bmq", [D]),
        ("g0", [D]), ("b0", [D]), ("g1", [D]), ("b1", [D]),
    ]:
        P[name] = nc.declare_dram_parameter(name, shape, F32, isOutput=False)
    for name, shape in [
        ("YT", [B, 2, 128, N]),
        ("WqT", [D, D]), ("WkT", [D, D]), ("WvT", [D, D]), ("WmixT", [D, D]),
        ("wi0T", [D, 4 * D]), ("wi1T", [D, 4 * D]), ("woT", [4 * D, D]),
        ("ET", [H, N, NL]), ("MT", [H, N, NL]),
    ]:
        P[name] = nc.declare_dram_parameter(name, shape, F16, isOutput=False)
    out_ext = nc.declare_dram_parameter("out", [B, NL, D], F32, isOutput=True)

    with tile.TileContext(nc) as tc:
        with tc.tile_pool(name="persist", bufs=1) as pp, \
             tc.tile_pool(name="ln", bufs=2) as lnp, \
             tc.tile_pool(name="enc", bufs=2) as encp, \
             tc.tile_pool(name="work", bufs=2) as wkp, \
             tc.tile_pool(name="ptp", bufs=2) as ptp, \
             tc.tile_pool(name="atp", bufs=2) as atp:

            # ---------- constants ----------
            idf = pp.tile([128, 128], F16, tag="idf")
            make_identity(nc, idf)
            ones_col = pp.tile([128, 1], F16, tag="ones_col")
            nc.vector.memset(ones_col, 1.0)
            mask_all = pp.tile([1, 4 * 128], F16, tag="mask_all")
            nc.vector.memset(mask_all, 0.0)
            for j in range(4):
                nc.vector.memset(mask_all[0:1, j * 128 + 32 * j:
                                          j * 128 + 32 * j + 32], 1.0)

            bcol = {}
            for name in ("bq", "bk"):
                t = pp.tile([128, 2], F32, tag=f"bcol_{name}")
                nc.sync.dma_start(out=t,
                                  in_=P[name][:].rearrange("(c p) -> p c", c=2))
                bcol[name] = t
            bcast = {}
            for name in ("g0", "b0", "g1", "b1", "bv", "bmq"):
                t = pp.tile([128, D], F32, tag=f"bc_{name}")
                ap = P[name][:].rearrange("(o d) -> o d", o=1)
                bap = bass.AP(tensor=ap.tensor, offset=ap.offset,
                              ap=[[0, 128], ap.ap[1]])
                nc.sync.dma_start(out=t, in_=bap)
                bcast[name] = t

            # ---------- weight DMA (host-pretransposed, fp16) ----------
            def load_w(hnd, rows, cols, tagp):
                """DRAM [rows, cols] -> rows//128 tiles of [128, cols]."""
                tiles = []
                for ri in range(rows // 128):
                    t = pp.tile([128, cols], F16, tag=f"{tagp}{ri}",
                                name=f"{tagp}{ri}")
                    nc.sync.dma_start(
                        out=t,
                        in_=hnd[:].rearrange("(t p) c -> t p c", p=128)[ri])
                    tiles.append(t)
                return tiles

            WqT = load_w(P["WqT"], D, D, "WqT")        # 2 x [128(d), 256(e)]
            WkT = load_w(P["WkT"], D, D, "WkT")
            WvT = load_w(P["WvT"], D, D, "WvT")
            WmixT = load_w(P["WmixT"], D, D, "WmixT")  # 2 x [128(e), 256(e')]
            wi0T = load_w(P["wi0T"], D, 4 * D, "wi0T")  # 2 x [128(e), 1024(u)]
            wi1T = load_w(P["wi1T"], D, 4 * D, "wi1T")
            woT = load_w(P["woT"], 4 * D, D, "woT")    # 8 x [128(u), 256(o)]

            # X / Y loads live in a scoped pool released after phase 2
            xyp = tc.alloc_tile_pool(name="xyp", bufs=1)
            x_n = []
            for b in range(B):
                t = xyp.tile([128, 2 * D], F32, tag=f"xload{b}",
                             name=f"xload{b}")
                nc.sync.dma_start(
                    out=t.rearrange("p (s d) -> p s d", s=2),
                    in_=P["Xs"][b].rearrange("(s p) d -> p s d", p=128))
                x_n.append(t)

            # enc tables for h=0 prefetch
            def load_enc(h):
                et = encp.tile([128, NKT * NL], F16, tag="ET")
                nc.sync.dma_start(
                    out=et.rearrange("p (kt t) -> p kt t", kt=NKT),
                    in_=P["ET"][h].rearrange("(kt p) t -> p kt t", p=128))
                mt = encp.tile([128, NKT * NL], F16, tag="MT")
                nc.sync.dma_start(
                    out=mt.rearrange("p (kt t) -> p kt t", kt=NKT),
                    in_=P["MT"][h].rearrange("(kt p) t -> p kt t", p=128))
                return et, mt

            enc_cur = load_enc(0)

            # yT load (host-pretransposed Y)
            yT = []
            for b in range(B):
                row = []
                for dd in range(2):
                    t = xyp.tile([128, N], F16, tag=f"yT{b}{dd}",
                                 name=f"yT{b}{dd}")
                    nc.sync.dma_start(out=t, in_=P["YT"][b][dd])
                    row.append(t)
                yT.append(row)

            # ---------- persistent SBUF ----------
            lnxT = [pp.tile([128, TOK], F16, tag=f"lnxT{i}", name=f"lnxT{i}") for i in range(2)]
            qsT = [pp.tile([64, TOK], F16, tag=f"qsT{i}", name=f"qsT{i}") for i in range(4)]
            qN = [pp.tile([128, D], F32, tag=f"qN{i}", name=f"qN{i}") for i in range(4)]
            kT = [[pp.tile([64, N], F16, tag=f"kT{b}{j}", name=f"kT{b}{j}") for j in range(4)]
                  for b in range(B)]
            vN = [pp.tile([128, NKT * D], F16, tag=f"vN{b}", name=f"vN{b}") for b in range(B)]
            mhT = [pp.tile([128, TOK], F16, tag=f"mhT{i}", name=f"mhT{i}") for i in range(2)]
            rcps = [pp.tile([1, NL], F16, tag=f"rcps{g}", name=f"rcps{g}")
                    for g in range(H * B)]
            hid = [pp.tile([128, D], F32, tag=f"hid{i}", name=f"hid{i}") for i in range(4)]
            hrT = [pp.tile([128, TOK], F16, tag=f"hrT{i}", name=f"hrT{i}") for i in range(2)]
            ffin = [pp.tile([128, TOK], F16, tag=f"ffin{i}", name=f"ffin{i}") for i in range(8)]

            with tc.tile_pool(name="psB", bufs=2, space="PSUM") as psB, \
                 tc.tile_pool(name="psT", bufs=2, space="PSUM") as psT:
                # ---------- phase 1: LN0, lnxT, Q ----------
                lnxb = []
                for b in range(B):
                    for s in range(2):
                        o = wkp.tile([128, D], F16, tag="lnxb")
                        _ln_apply(nc, lnp, x_n[b][:, s * D:(s + 1) * D],
                                  bcast["g0"], bcast["b0"], o)
                        lnxb.append(o)                   # tt = b*2 + s
                        tt = b * 2 + s
                        for dd in range(2):
                            pst = psT.tile([128, 128], F16, tag="tr")
                            nc.tensor.transpose(
                                pst, o[:, dd * 128:(dd + 1) * 128], idf)
                            nc.scalar.copy(
                                lnxT[dd][:, tt * 128:(tt + 1) * 128], pst)

                # qsT: [de, tok] fp16 (bias bq via activation)
                for ee in range(2):
                    ps = psB.tile([128, TOK], F32, tag="big")
                    for dd in range(2):
                        nc.tensor.matmul(ps, WqT[dd][:, ee * 128:(ee + 1) * 128],
                                         lnxT[dd], start=(dd == 0),
                                         stop=(dd == 1))
                    for jj in range(2):
                        sl = slice(jj * 64, (jj + 1) * 64)
                        nc.scalar.activation(qsT[ee * 2 + jj], ps[sl, :],
                                             AF.Identity,
                                             bias=bcol["bq"][sl, ee:ee + 1])
                # qN: [tok, de] f32 residual incl. bq + bmix (bmq)
                for tt in range(4):
                    ps = psB.tile([128, TOK], F32, tag="big")
                    for dd in range(2):
                        nc.tensor.matmul(ps[:, 0:D],
                                         lnxT[dd][:, tt * 128:(tt + 1) * 128],
                                         WqT[dd], start=(dd == 0),
                                         stop=(dd == 1))
                    nc.vector.tensor_tensor(qN[tt], ps[:, 0:D], bcast["bmq"],
                                            AX.add)

                # ---------- phase 2: K^T and V ----------
                for b in range(B):
                    for ee in range(2):
                        for ch in range(4):
                            ps = psB.tile([128, TOK], F32, tag="big")
                            sl = slice(ch * 512, (ch + 1) * 512)
                            for dd in range(2):
                                nc.tensor.matmul(
                                    ps, WkT[dd][:, ee * 128:(ee + 1) * 128],
                                    yT[b][dd][:, sl], start=(dd == 0),
                                    stop=(dd == 1))
                            for jj in range(2):
                                psl = slice(jj * 64, (jj + 1) * 64)
                                nc.scalar.activation(
                                    kT[b][ee * 2 + jj][:, sl], ps[psl, :],
                                    AF.Identity,
                                    bias=bcol["bk"][psl, ee:ee + 1])
                    for kt in range(NKT):
                        ps = psB.tile([128, TOK], F32, tag="big")
                        for dd in range(2):
                            nc.tensor.matmul(
                                ps[:, 0:D],
                                yT[b][dd][:, kt * 128:(kt + 1) * 128],
                                WvT[dd], start=(dd == 0), stop=(dd == 1))
                        nc.vector.tensor_tensor(
                            vN[b][:, kt * D:(kt + 1) * D], ps[:, 0:D],
                            bcast["bv"], AX.add)

            xyp.release()

            # ---------- phase 3: attention ----------
            with tc.tile_pool(name="psS", bufs=4, space="PSUM") as psS, \
                 tc.tile_pool(name="psM", bufs=2, space="PSUM") as psM, \
                 tc.tile_pool(name="psD", bufs=2, space="PSUM") as psD:

                def denmh_thunks(g, ew, aw):
                    h, b = divmod(g, B)
                    ee, r = h // 4, 32 * (h % 4)
                    ps_d = psD.tile([1, NL], F32, tag="d")
                    ps_m = psM.tile([DS, NL], F32, tag="m")
                    th = []
                    for kt in range(NKT):
                        th.append(lambda kt=kt: nc.tensor.matmul(
                            ps_d, ones_col, ew[:, kt * NL:(kt + 1) * NL],
                            start=(kt == 0), stop=(kt == NKT - 1)))
                    for kt in range(NKT):
                        th.append(lambda kt=kt: nc.tensor.matmul(
                            ps_m, vN[b][:, kt * D + h * DS:kt * D + (h + 1) * DS],
                            aw[:, kt * NL:(kt + 1) * NL],
                            start=(kt == 0), stop=(kt == NKT - 1)))

                    def fin():
                        rcp = lnp.tile([1, NL], F32, tag="rcp")
                        nc.vector.reciprocal(rcp, ps_d)
                        nc.vector.tensor_copy(rcps[g], rcp)
                        nc.scalar.copy(
                            mhT[ee][r:r + DS, b * NL:(b + 1) * NL], ps_m)
                    th.append(fin)
                    return th

                prev = []
                for h in range(H):
                    et, mt = enc_cur
                    if h + 1 < H:
                        enc_cur = load_enc(h + 1)
                    j, r2 = h // 2, 32 * (h % 2)
                    for b in range(B):
                        g = h * B + b
                        ptw = ptp.tile([128, NKT * NL], F16, tag="pt")
                        aw = atp.tile([128, NKT * NL], F16, tag="at")
                        ew = ptw  # ET multiply happens in place after at
                        for kp in range(8):
                            ps = psS.tile([128, 2 * NL], F32, tag="s")
                            for jj in range(2):
                                kt = 2 * kp + jj
                                nc.tensor.matmul(
                                    ps[:, jj * NL:(jj + 1) * NL],
                                    kT[b][j][r2:r2 + DS,
                                             kt * 128:(kt + 1) * 128],
                                    qsT[j][r2:r2 + DS, b * NL:(b + 1) * NL],
                                    start=True, stop=True)
                            sl = slice(kp * 2 * NL, (kp + 1) * 2 * NL)
                            nc.scalar.activation(ptw[:, sl], ps, AF.Exp,
                                                 scale=1.0 / 16.0)
                            nc.vector.tensor_tensor(aw[:, sl], ptw[:, sl],
                                                    mt[:, sl], AX.mult)
                            nc.vector.tensor_tensor(ptw[:, sl], ptw[:, sl],
                                                    et[:, sl], AX.mult)
                            # interleave den/MH of previous group
                            if kp >= 2 and prev:
                                for _ in range(6):
                                    if prev:
                                        prev.pop(0)()
                        while prev:
                            prev.pop(0)()
                        prev = denmh_thunks(g, ew, aw)
                for t in prev:
                    t()

            with tc.tile_pool(name="psC", bufs=2, space="PSUM") as psC, \
                 tc.tile_pool(name="psT2", bufs=2, space="PSUM") as psT2:
                # ---------- phase 4: 1/den expand, mhs, mix, hid ----------
                rb = [wkp.tile([128, TOK], F16, tag=f"rb{i}", name=f"rb{i}")
                      for i in range(2)]
                for ee in range(2):
                    for b in range(B):
                        ps = psT2.tile([128, NL], F32, tag="sm")
                        for hh in range(4):
                            g = (ee * 4 + hh) * B + b
                            nc.tensor.matmul(
                                ps, mask_all[0:1, hh * 128:(hh + 1) * 128],
                                rcps[g], start=(hh == 0), stop=(hh == 3))
                        nc.scalar.copy(rb[ee][:, b * NL:(b + 1) * NL], ps)
                mhsT = [wkp.tile([128, TOK], F16, tag=f"mhsT{i}", name=f"mhsT{i}")
                        for i in range(2)]
                for ee in range(2):
                    nc.vector.tensor_tensor(mhsT[ee], mhT[ee], rb[ee], AX.mult)
                for tb in range(4):
                    ps = psC.tile([128, TOK], F32, tag="big")
                    for ee in range(2):
                        nc.tensor.matmul(ps[:, 0:D],
                                         mhsT[ee][:, tb * 128:(tb + 1) * 128],
                                         WmixT[ee], start=(ee == 0),
                                         stop=(ee == 1))
                    nc.vector.tensor_tensor(hid[tb], ps[:, 0:D], qN[tb],
                                            AX.add)

                # ---------- phase 5: LN1 + FFN + residual out ----------
                for tb in range(4):
                    hrb = wkp.tile([128, D], F16, tag="hrb")
                    _ln_apply(nc, lnp, hid[tb], bcast["g1"], bcast["b1"], hrb)
                    for dd in range(2):
                        pst = psT2.tile([128, 128], F16, tag="tr2")
                        nc.tensor.transpose(
                            pst, hrb[:, dd * 128:(dd + 1) * 128], idf)
                        nc.scalar.copy(hrT[dd][:, tb * 128:(tb + 1) * 128],
                                       pst)
                for ub in range(8):
                    ps0 = psC.tile([128, TOK], F32, tag="big")
                    ps1 = psC.tile([128, TOK], F32, tag="big")
                    for dd in range(2):
                        nc.tensor.matmul(ps0,
                                         wi0T[dd][:, ub * 128:(ub + 1) * 128],
                                         hrT[dd], start=(dd == 0),
                                         stop=(dd == 1))
                    for dd in range(2):
                        nc.tensor.matmul(ps1,
                                         wi1T[dd][:, ub * 128:(ub + 1) * 128],
                                         hrT[dd], start=(dd == 0),
                                         stop=(dd == 1))
                    gt = wkp.tile([128, TOK], F16, tag="gelu")
                    nc.scalar.activation(gt, ps0, gelu_af)
                    nc.vector.tensor_tensor(ffin[ub], gt, ps1, AX.mult)
                for tb in range(4):
                    ps = psC.tile([128, TOK], F32, tag="big")
                    for ku in range(8):
                        nc.tensor.matmul(ps[:, 0:D],
                                         ffin[ku][:, tb * 128:(tb + 1) * 128],
                                         woT[ku], start=(ku == 0),
                                         stop=(ku == 7))
                    o = wkp.tile([128, D], F32, tag="outN")
                    nc.vector.tensor_tensor(o, ps[:, 0:D], hid[tb], AX.add)
                    nc.sync.dma_start(
                        out=out_ext[tb // 2].rearrange(
                            "(s p) d -> s p d", p=128)[tb % 2],
                        in_=o)
    nc.finalize()
    return nc


def prepare_in_maps(inputs):
    f16 = np.float16
    X = np.asarray(inputs["X"], np.float32)
    Yf = np.asarray(inputs["Y"], np.float32)
    add_enc = np.asarray(inputs["add_enc"], np.float32)
    mult_enc = np.asarray(inputs["mult_enc"], np.float32)
    Ef = np.exp(add_enc / 16.0)
    Mf = mult_enc * Ef

    common = {k: np.asarray(inputs[k], np.float32)
              for k in ("bq", "bk", "bv", "g0", "b0", "g1", "b1")}
    common["bmq"] = (np.asarray(inputs["bq"], np.float32)
                     + np.asarray(inputs["bmix"], np.float32))
    for k, v in (("WqT", inputs["Wq"]), ("WkT", inputs["Wk"]),
                 ("WvT", inputs["Wv"]), ("WmixT", inputs["Wmix"]),
                 ("wi0T", inputs["wi0"]), ("wi1T", inputs["wi1"]),
                 ("woT", inputs["wo"])):
        common[k] = np.ascontiguousarray(
            np.asarray(v, np.float32).T).astype(f16)
    common["YT"] = np.ascontiguousarray(
        Yf.transpose(0, 2, 1)).reshape(B, 2, 128, N).astype(f16)

    in_maps = []
    for c in range(NCORES):
        sl = slice(c * NL, (c + 1) * NL)
        m = dict(common)
        m["Xs"] = np.ascontiguousarray(X[:, sl, :])
        m["ET"] = np.ascontiguousarray(
            Ef[:, sl, :].transpose(0, 2, 1)).astype(f16)
        m["MT"] = np.ascontiguousarray(
            Mf[:, sl, :].transpose(0, 2, 1)).astype(f16)
        in_maps.append(m)
    return in_maps


def kernel(**inputs):
    in_maps = prepare_in_maps(inputs)
    nc = build_kernel()
    res = run_bass_kernel_spmd(nc, in_maps, list(range(NCORES)))
    out = np.empty((B, N, D), np.float32)
    for c in range(NCORES):
        out[:, c * NL:(c + 1) * NL, :] = res.results[c]["out"]
    return out


if __name__ == "__main__":
    nc = build_kernel()
    print("build OK")


# revision 51
# speedup vs baseline: 2.6912x; 1.2354x over previous
"""Trainium2 Bass kernel for nn_MAB_17471926960685 (dense_transformer).

Sharding: token-parallel over N. Each of 8 cores takes a 256-token slice of N
(both batches); K/V are computed replicated from the full Y. No collectives.

v2 design notes:
  - All weights and Y are pre-transposed on host (no DMA transposes).
  - add_enc is folded on host: ET = exp(add_enc/16), MT = mult_enc * ET.
    Scores then need only exp(QK/16) on ACT; the enc-add matmul pass is gone.
  - Attention per (head, batch) group g: PE computes 16 score matmuls
    (keys on partitions, tokens free); ACT evacuates PSUM pairs with
    exp(scale=1/16) to fp16; DVE forms at = MT*pt (PV weights) and
    ept = ET*pt (denominator integrand); PE then accumulates den (ones
    matmul) and MH (V matmul) for the PREVIOUS group, interleaved between
    score pairs so the PE never waits on the ACT/DVE chain.
  - fp16 everywhere for 2-byte tensors (better mantissa than bf16, same
    PE/DVE speed); f32 for residual/LN paths.
"""

import math
import sys

import numpy as np
import ml_dtypes

sys.path.insert(0, "/opt/trn_rl_repo")

import concourse.bass as bass
import concourse.mybir as mybir
import concourse.tile as tile
from concourse import bacc
from concourse.masks import make_identity
from concourse.bass_utils import run_bass_kernel_spmd

B, N, D, H = 2, 2048, 256, 8
DS = D // H          # 32
NCORES = 8
NL = N // NCORES     # 256 tokens per core per batch
TOK = B * NL         # 512 tokens per core
NKT = N // 128       # 16 key tiles
EPS = 1e-5
F32 = mybir.dt.float32
F16 = mybir.dt.float16
AX = mybir.AluOpType
AF = mybir.ActivationFunctionType


def _ln_apply(nc, pool, x_ap, g_bc, b_bc, out_ap):
    """LayerNorm rows of x_ap [128, D] -> out_ap (any dtype)."""
    stats = pool.tile([128, 6], F32, tag="ln_stats")
    mv = pool.tile([128, 2], F32, tag="ln_mv")
    nc.vector.bn_stats(out=stats, in_=x_ap)
    nc.vector.bn_aggr(out=mv, in_=stats)
    eps_t = pool.tile([128, 1], F32, tag="ln_eps")
    nc.vector.memset(eps_t, EPS)
    std = pool.tile([128, 1], F32, tag="ln_std")
    nc.scalar.activation(std, mv[:, 1:2], AF.Sqrt, bias=eps_t)
    rstd = pool.tile([128, 1], F32, tag="ln_rstd")
    nc.vector.reciprocal(rstd, std)
    xn = pool.tile([128, D], F32, tag="ln_xn")
    nc.vector.tensor_scalar(xn, x_ap, mv[:, 0:1], rstd, AX.subtract, AX.mult)
    nc.vector.tensor_tensor(xn, xn, g_bc, AX.mult)
    nc.vector.tensor_tensor(out_ap, xn, b_bc, AX.add)


def build_kernel(gelu_af=AF.Gelu_apprx_tanh):
    nc = bacc.Bacc()
    P = {}
    for name, shape in [
        ("Xs", [B, NL, D]),
        ("bq", [D]), ("bk", [D]), ("bv", [D]), ("bmq", [D]),
        ("g0", [D]), ("b0", [D]), ("g1", [D]), ("b1", [D]),
    ]:
        P[name] = nc.declare_dram_parameter(name, shape, F32, isOutput=False)
    for name, shape in [
        ("YT", [B, 2, 128, N]),
        ("WqT", [D, D]), ("WkT", [D, D]), ("WvT", [D, D]), ("WmixT", [D, D]),
        ("wi0T", [D, 4 * D]), ("wi1T", [D, 4 * D]), ("woT", [4 * D, D]),
        ("ET", [H, N, NL]), ("MT", [H, N, NL]), ("bvh", [D]),
    ]:
        P[name] = nc.declare_dram_parameter(name, shape, F16, isOutput=False)
    out_ext = nc.declare_dram_parameter("out", [B, NL, D], F32, isOutput=True)

    with tile.TileContext(nc) as tc:
        with tc.tile_pool(name="persist", bufs=1) as pp, \
             tc.tile_pool(name="ln", bufs=2) as lnp, \
             tc.tile_pool(name="enc", bufs=2) as encp, \
             tc.tile_pool(name="work", bufs=2) as wkp, \
             tc.tile_pool(name="ptp", bufs=2) as ptp, \
             tc.tile_pool(name="atp", bufs=2) as atp:

            # ---------- constants ----------
            idf = pp.tile([128, 128], F16, tag="idf")
            make_identity(nc, idf)
            ones_col = pp.tile([128, 1], F16, tag="ones_col")
            nc.vector.memset(ones_col, 1.0)
            ones_row = pp.tile([1, 128], F16, tag="ones_row")
            nc.vector.memset(ones_row, 1.0)
            bv_row = pp.tile([1, D], F16, tag="bv_row")
            nc.sync.dma_start(out=bv_row,
                              in_=P["bvh"][:].rearrange("(o d) -> o d", o=1))
            ones_row32 = pp.tile([1, DS], F16, tag="ones_row32")
            nc.vector.memset(ones_row32, 1.0)

            bcol = {}
            for name in ("bq", "bk"):
                t = pp.tile([128, 2], F32, tag=f"bcol_{name}")
                nc.sync.dma_start(out=t,
                                  in_=P[name][:].rearrange("(c p) -> p c", c=2))
                bcol[name] = t
            bcast = {}
            for name in ("g0", "b0", "g1", "b1", "bv", "bmq"):
                t = pp.tile([128, D], F32, tag=f"bc_{name}")
                ap = P[name][:].rearrange("(o d) -> o d", o=1)
                bap = bass.AP(tensor=ap.tensor, offset=ap.offset,
                              ap=[[0, 128], ap.ap[1]])
                nc.sync.dma_start(out=t, in_=bap)
                bcast[name] = t

            # ---------- weight DMA (host-pretransposed, fp16) ----------
            def load_w(hnd, rows, cols, tagp):
                """DRAM [rows, cols] -> rows//128 tiles of [128, cols]."""
                tiles = []
                for ri in range(rows // 128):
                    t = pp.tile([128, cols], F16, tag=f"{tagp}{ri}",
                                name=f"{tagp}{ri}")
                    nc.sync.dma_start(
                        out=t,
                        in_=hnd[:].rearrange("(t p) c -> t p c", p=128)[ri])
                    tiles.append(t)
                return tiles

            # X / Y loads live in a scoped pool released after phase 2.
            # DMA issue order matters for startup: X + WqT first (LN0/Q),
            # then Y + WkT/WvT (phase 2), then enc tables for h=0.
            # FFN/mix weights are issued after the attention emission.
            xyp = tc.alloc_tile_pool(name="xyp", bufs=1)
            x_n = []
            for b in range(B):
                t = xyp.tile([128, 2 * D], F32, tag=f"xload{b}",
                             name=f"xload{b}")
                nc.sync.dma_start(
                    out=t.rearrange("p (s d) -> p s d", s=2),
                    in_=P["Xs"][b].rearrange("(s p) d -> p s d", p=128))
                x_n.append(t)

            WqT = load_w(P["WqT"], D, D, "WqT")        # 2 x [128(d), 256(e)]
            WkT = load_w(P["WkT"], D, D, "WkT")
            WvT = load_w(P["WvT"], D, D, "WvT")

            # yT load (host-pretransposed Y)
            yT = []
            for b in range(B):
                row = []
                for dd in range(2):
                    t = xyp.tile([128, N], F16, tag=f"yT{b}{dd}",
                                 name=f"yT{b}{dd}")
                    nc.sync.dma_start(out=t, in_=P["YT"][b][dd])
                    row.append(t)
                yT.append(row)

            # enc tables for h=0 prefetch
            def load_enc(h):
                et = encp.tile([128, NKT * NL], F16, tag="ET")
                nc.sync.dma_start(
                    out=et.rearrange("p (kt t) -> p kt t", kt=NKT),
                    in_=P["ET"][h].rearrange("(kt p) t -> p kt t", p=128))
                mt = encp.tile([128, NKT * NL], F16, tag="MT")
                nc.sync.dma_start(
                    out=mt.rearrange("p (kt t) -> p kt t", kt=NKT),
                    in_=P["MT"][h].rearrange("(kt p) t -> p kt t", p=128))
                return et, mt

            enc_cur = load_enc(0)

            # mix/FFN weights: needed only in phases 4-5, so issued after
            # everything the early phases depend on
            WmixT = load_w(P["WmixT"], D, D, "WmixT")  # 2 x [128(e), 256(e')]
            wi0T = load_w(P["wi0T"], D, 4 * D, "wi0T")  # 2 x [128(e), 1024(u)]
            wi1T = load_w(P["wi1T"], D, 4 * D, "wi1T")
            woT = load_w(P["woT"], 4 * D, D, "woT")    # 8 x [128(u), 256(o)]

            # ---------- persistent SBUF ----------
            lnxT = [pp.tile([128, TOK], F16, tag=f"lnxT{i}", name=f"lnxT{i}") for i in range(2)]
            qsT = [pp.tile([64, TOK], F16, tag=f"qsT{i}", name=f"qsT{i}") for i in range(4)]
            qN = [pp.tile([128, D], F32, tag=f"qN{i}", name=f"qN{i}") for i in range(4)]
            kT = [[pp.tile([64, N], F16, tag=f"kT{b}{j}", name=f"kT{b}{j}") for j in range(4)]
                  for b in range(B)]
            vN = [pp.tile([128, NKT * D], F16, tag=f"vN{b}", name=f"vN{b}") for b in range(B)]
            mhsT = [pp.tile([128, TOK], F16, tag=f"mhsT{i}", name=f"mhsT{i}")
                    for i in range(2)]
            hid = [pp.tile([128, D], F32, tag=f"hid{i}", name=f"hid{i}") for i in range(4)]
            hrT = [pp.tile([128, TOK], F16, tag=f"hrT{i}", name=f"hrT{i}") for i in range(2)]
            ffin = [pp.tile([128, TOK], F16, tag=f"ffin{i}", name=f"ffin{i}") for i in range(8)]

            psB = tc.alloc_tile_pool(name="psB", bufs=2, space="PSUM")
            with tc.tile_pool(name="psT", bufs=2, space="PSUM") as psT:
                # ---------- phase 1: LN0, lnxT, Q ----------
                lnxb = []
                for b in range(B):
                    for s in range(2):
                        o = wkp.tile([128, D], F16, tag="lnxb")
                        _ln_apply(nc, lnp, x_n[b][:, s * D:(s + 1) * D],
                                  bcast["g0"], bcast["b0"], o)
                        lnxb.append(o)                   # tt = b*2 + s
                for tt in range(4):
                    pst = psT.tile([128, 256], F16, tag="tr")
                    for dd in range(2):
                        nc.tensor.transpose(
                            pst[:, dd * 128:(dd + 1) * 128],
                            lnxb[tt][:, dd * 128:(dd + 1) * 128], idf)
                    for dd in range(2):
                        nc.scalar.copy(
                            lnxT[dd][:, tt * 128:(tt + 1) * 128],
                            pst[:, dd * 128:(dd + 1) * 128])

                # qsT: [de, tok] fp16 (bias bq via activation)
                for ee in range(2):
                    ps = psB.tile([128, TOK], F32, tag="big")
                    for dd in range(2):
                        nc.tensor.matmul(ps, WqT[dd][:, ee * 128:(ee + 1) * 128],
                                         lnxT[dd], start=(dd == 0),
                                         stop=(dd == 1))
                    for jj in range(2):
                        sl = slice(jj * 64, (jj + 1) * 64)
                        nc.scalar.activation(qsT[ee * 2 + jj], ps[sl, :],
                                             AF.Identity,
                                             bias=bcol["bq"][sl, ee:ee + 1])
                # qN: [tok, de] f32 residual incl. bq + bmix (bmq)
                for tt in range(4):
                    ps = psB.tile([128, TOK], F32, tag="big")
                    for dd in range(2):
                        nc.tensor.matmul(ps[:, 0:D],
                                         lnxT[dd][:, tt * 128:(tt + 1) * 128],
                                         WqT[dd], start=(dd == 0),
                                         stop=(dd == 1))
                    nc.vector.tensor_tensor(qN[tt], ps[:, 0:D], bcast["bmq"],
                                            AX.add)

                # ---------- phase 2: K^T and V (interleaved). Batch 0 is
                # emitted here; batch 1 is deferred into the first attention
                # group's interleave slots so attention starts sooner. ------
                def k_chunk(b, ee, ch):
                    ps = psB.tile([128, TOK], F32, tag="big", name="k_ps")
                    sl = slice(ch * 512, (ch + 1) * 512)
                    for dd in range(2):
                        nc.tensor.matmul(
                            ps, WkT[dd][:, ee * 128:(ee + 1) * 128],
                            yT[b][dd][:, sl], start=(dd == 0), stop=(dd == 1))
                    for jj in range(2):
                        psl = slice(jj * 64, (jj + 1) * 64)
                        if jj == 0:
                            nc.scalar.activation(
                                kT[b][ee * 2 + jj][:, sl], ps[psl, :],
                                AF.Identity, bias=bcol["bk"][psl, ee:ee + 1])
                        else:
                            nc.vector.tensor_scalar(
                                kT[b][ee * 2 + jj][:, sl], ps[psl, :],
                                bcol["bk"][psl, ee:ee + 1], None, AX.add)

                def v_pair(b, kp):
                    for kt in (2 * kp, 2 * kp + 1):
                        psv = psB.tile([128, TOK], F32, tag="big", name="v_ps")
                        for dd in range(2):
                            nc.tensor.matmul(
                                psv[:, 0:D],
                                yT[b][dd][:, kt * 128:(kt + 1) * 128],
                                WvT[dd], start=(dd == 0), stop=(dd == 1))
                        nc.vector.tensor_tensor(
                            vN[b][:, kt * D:(kt + 1) * D],
                            psv[:, 0:D], bcast["bv"], AX.add)

                for ee in range(2):
                    for ch in range(4):
                        k_chunk(0, ee, ch)
                        v_pair(0, 4 * ee + ch)

            kv_b1 = [(lambda ee=ee, ch=ch: k_chunk(1, ee, ch))
                     for ee in range(2) for ch in range(4)]
            kv_b1 += [(lambda kp=kp: v_pair(1, kp)) for kp in range(8)]

            # ---------- phase 3: attention ----------
            # one [64, 2*NL] PSUM tile per head holds both the MH accumulator
            # (rows 0:32) and the den accumulator row (row 32), so psS can
            # triple-buffer within the 8 PSUM banks
            with tc.tile_pool(name="psS", bufs=2, space="PSUM") as psS, \
                 tc.tile_pool(name="psM", bufs=2, space="PSUM") as psM:

                mh_ps = [None]  # per-h accumulator, allocated at b == 0

                def denmh_thunks(g, ew, aw):
                    h, b = divmod(g, B)
                    ee, r = h // 4, 32 * (h % 4)
                    if b == 0:
                        mh_ps[0] = psM.tile([96, 2 * NL], F32, tag="m",
                                            name="mh_acc")
                    ps_m = mh_ps[0]
                    csl = slice(b * NL, (b + 1) * NL)
                    th = []
                    for kt in range(NKT):
                        th.append(lambda kt=kt: nc.tensor.matmul(
                            ps_m[32:33, csl], ones_col,
                            ew[:, kt * NL:(kt + 1) * NL],
                            start=(kt == 0), stop=(kt == NKT - 1)))
                    for kt in range(NKT):
                        th.append(lambda kt=kt: nc.tensor.matmul(
                            ps_m[0:DS, csl],
                            vN[b][:, kt * D + h * DS:kt * D + (h + 1) * DS],
                            aw[:, kt * NL:(kt + 1) * NL],
                            start=(kt == 0), stop=(kt == NKT - 1)))

                    def fin():
                        if b == 0:
                            return
                        # 1/den for both batches, broadcast onto 32 rows via
                        # a rank-1 matmul into rows 64:96, then scale MH and
                        # write the mhsT strip in one DVE op
                        rcp16 = lnp.tile([1, 2 * NL], F16, tag="rcp16")
                        with nc.allow_low_precision(
                                reason="1/den feeds an fp16 multiply"):
                            nc.vector.reciprocal(rcp16, ps_m[32:33, :])
                        nc.tensor.matmul(ps_m[64:96, :], ones_row32, rcp16,
                                         start=True, stop=True)
                        rbs = lnp.tile([DS, 2 * NL], F16, tag="rbs")
                        nc.scalar.copy(rbs, ps_m[64:96, :])
                        nc.vector.tensor_tensor(
                            mhsT[ee][r:r + DS, :], ps_m[0:DS, :],
                            rbs, AX.mult)
                    th.append(fin)
                    return th

                prev = kv_b1
                for h in range(H):
                    et, mt = enc_cur
                    if h + 1 < H:
                        enc_cur = load_enc(h + 1)
                    j, r2 = h // 2, 32 * (h % 2)
                    for b in range(B):
                        g = h * B + b
                        ptw = ptp.tile([128, NKT * NL], F16, tag="pt")
                        aw = atp.tile([128, NKT * NL], F16, tag="at")
                        ew = ptw  # ET multiply happens in place after at
                        for q in range(4):
                            ps = psS.tile([128, 4 * NL], F32, tag="s")
                            for jj in range(4):
                                kt = 4 * q + jj
                                nc.tensor.matmul(
                                    ps[:, jj * NL:(jj + 1) * NL],
                                    kT[b][j][r2:r2 + DS,
                                             kt * 128:(kt + 1) * 128],
                                    qsT[j][r2:r2 + DS, b * NL:(b + 1) * NL],
                                    start=True, stop=True)
                            sl = slice(q * 4 * NL, (q + 1) * 4 * NL)
                            nc.scalar.activation(ptw[:, sl], ps, AF.Exp,
                                                 scale=1.0 / 16.0)
                            # ept = ET*pt in place; then at = mult*ept
                            nc.vector.tensor_tensor(ptw[:, sl], ptw[:, sl],
                                                    et[:, sl], AX.mult)
                            aeng = nc.gpsimd if q == 3 else nc.vector
                            aeng.tensor_tensor(aw[:, sl], ptw[:, sl],
                                               mt[:, sl], AX.mult)
                            # interleave den/MH of previous group
                            if prev:
                                for _ in range(9):
                                    if prev:
                                        prev.pop(0)()
                        while prev:
                            prev.pop(0)()
                        prev = denmh_thunks(g, ew, aw)
                for t in prev:
                    t()
            psB.release()
            xyp.release()

            with tc.tile_pool(name="psC", bufs=2, space="PSUM") as psC, \
                 tc.tile_pool(name="psT2", bufs=2, space="PSUM") as psT2:
                # ---------- phase 4: mix, hid ----------
                for tb in range(4):
                    ps = psC.tile([128, TOK], F32, tag="big")
                    for ee in range(2):
                        nc.tensor.matmul(ps[:, 0:D],
                                         mhsT[ee][:, tb * 128:(tb + 1) * 128],
                                         WmixT[ee], start=(ee == 0),
                                         stop=(ee == 1))
                    nc.vector.tensor_tensor(hid[tb], ps[:, 0:D], qN[tb],
                                            AX.add)

                # ---------- phase 5: LN1 + FFN + residual out ----------
                for tb in range(4):
                    hrb = wkp.tile([128, D], F16, tag="hrb")
                    _ln_apply(nc, lnp, hid[tb], bcast["g1"], bcast["b1"], hrb)
                    pst = psT2.tile([128, 256], F16, tag="tr2")
                    for dd in range(2):
                        nc.tensor.transpose(
                            pst[:, dd * 128:(dd + 1) * 128],
                            hrb[:, dd * 128:(dd + 1) * 128], idf)
                    for dd in range(2):
                        nc.scalar.copy(hrT[dd][:, tb * 128:(tb + 1) * 128],
                                       pst[:, dd * 128:(dd + 1) * 128])
                for ub in range(8):
                    ps0 = psC.tile([128, TOK], F32, tag="big")
                    ps1 = psC.tile([128, TOK], F32, tag="big")
                    for dd in range(2):
                        nc.tensor.matmul(ps0,
                                         wi0T[dd][:, ub * 128:(ub + 1) * 128],
                                         hrT[dd], start=(dd == 0),
                                         stop=(dd == 1))
                    for dd in range(2):
                        nc.tensor.matmul(ps1,
                                         wi1T[dd][:, ub * 128:(ub + 1) * 128],
                                         hrT[dd], start=(dd == 0),
                                         stop=(dd == 1))
                    gt = wkp.tile([128, TOK], F16, tag="gelu")
                    nc.scalar.activation(gt, ps0, gelu_af)
                    nc.vector.tensor_tensor(ffin[ub], gt, ps1, AX.mult)
                for tb in range(4):
                    ps = psC.tile([128, TOK], F32, tag="big")
                    for ku in range(8):
                        nc.tensor.matmul(ps[:, 0:D],
                                         ffin[ku][:, tb * 128:(tb + 1) * 128],
                                         woT[ku], start=(ku == 0),
                                         stop=(ku == 7))
                    o = wkp.tile([128, D], F32, tag="outN")
                    nc.vector.tensor_tensor(o, ps[:, 0:D], hid[tb], AX.add)
                    nc.sync.dma_start(
                        out=out_ext[tb // 2].rearrange(
                            "(s p) d -> s p d", p=128)[tb % 2],
                        in_=o)
    nc.finalize()
    return nc


def prepare_in_maps(inputs):
    f16 = np.float16
    X = np.asarray(inputs["X"], np.float32)
    Yf = np.asarray(inputs["Y"], np.float32)
    add_enc = np.asarray(inputs["add_enc"], np.float32)
    mult_enc = np.asarray(inputs["mult_enc"], np.float32)
    Ef = np.exp(add_enc / 16.0)
    Mf = mult_enc

    common = {k: np.asarray(inputs[k], np.float32)
              for k in ("bq", "bk", "bv", "g0", "b0", "g1", "b1")}
    common["bvh"] = np.asarray(inputs["bv"], np.float32).astype(f16)
    common["bmq"] = (np.asarray(inputs["bq"], np.float32)
                     + np.asarray(inputs["bmix"], np.float32))
    for k, v in (("WqT", inputs["Wq"]), ("WkT", inputs["Wk"]),
                 ("WvT", inputs["Wv"]), ("WmixT", inputs["Wmix"]),
                 ("wi0T", inputs["wi0"]), ("wi1T", inputs["wi1"]),
                 ("woT", inputs["wo"])):
        common[k] = np.ascontiguousarray(
            np.asarray(v, np.float32).T).astype(f16)
    common["YT"] = np.ascontiguousarray(
        Yf.transpose(0, 2, 1)).reshape(B, 2, 128, N).astype(f16)

    in_maps = []
    for c in range(NCORES):
        sl = slice(c * NL, (c + 1) * NL)
        m = dict(common)
        m["Xs"] = np.ascontiguousarray(X[:, sl, :])
        m["ET"] = np.ascontiguousarray(
            Ef[:, sl, :].transpose(0, 2, 1)).astype(f16)
        m["MT"] = np.ascontiguousarray(
            Mf[:, sl, :].transpose(0, 2, 1)).astype(f16)
        in_maps.append(m)
    return in_maps


def kernel(**inputs):
    in_maps = prepare_in_maps(inputs)
    nc = build_kernel()
    res = run_bass_kernel_spmd(nc, in_maps, list(range(NCORES)))
    out = np.empty((B, N, D), np.float32)
    for c in range(NCORES):
        out[:, c * NL:(c + 1) * NL, :] = res.results[c]["out"]
    return out


if __name__ == "__main__":
    nc = build_kernel()
    print("build OK")
